# revision 1
# baseline (speedup 1.0000x reference)
"""MoE (8 routed experts, top-2, + shared expert) on 8 TRN2 NeuronCores.

Strategy: expert-parallel with load balancing. Host computes the gate
(fp32 numpy, mirroring the reference), then packs work into a static
SPMD kernel with three column groups per core:

  R (a cols):  expert k's first min(c_k, a) tokens  (scaled by cw)
  S (TS cols): a 1/8 token-slice of the shared expert (unscaled)
  O (b cols):  an "overflow" slot bound to ANY expert — the host
               splits experts whose token count exceeds `a` into
               <=b-col pieces and bin-packs them over the 8 O slots.

All three groups are computed by ONE merged job: each phase-A i-pass
runs the O, S and R sub-passes back to back (and each phase-B do-pass
runs R, S, O), so all three weight sets stream from HBM evenly across
the whole kernel (~150 GB/s) instead of bursting past the per-core
wire rate in a short dedicated phase.

Sizes (a, b) are solved per-input to minimize modeled per-core PE
time, cutting the padding a single max-capacity job would pay.

All device tensors are pre-arranged on host into partition-major
layouts so every DMA is contiguous per partition: matmul lhsT/rhs
always have the contraction dim chunked as [pi=128, po, free].
"""

import numpy as np
import ml_dtypes

import concourse.mybir as mybir
from concourse import bacc
from concourse.tile import TileContext
from concourse import bass_utils

BF16 = mybir.dt.bfloat16
F32 = mybir.dt.float32

D = 2048          # model dim
I = 1408          # expert inter dim
E = 8             # routed experts
TOPK = 2
N_CORES = 8
DPO = D // 128    # 16 chunks of the model dim
IPO = I // 128    # 11 chunks of the inter dim

_BUILD_CACHE = {}


def _c_blocks(C):
    """Split C columns into equal-ish blocks <= 512."""
    nb = -(-C // 512)
    per = -(-C // (nb * 128)) * 128
    blocks = []
    off = 0
    while off < C:
        w = min(per, C - off)
        blocks.append((off, w))
        off += w
    return blocks


# PE cost model, measured on hardware: a matmul of W columns costs
# ~W/2.4 + 2.5 ns when interleaved with larger matmuls (the ~34.5ns
# instruction-issue floor only binds for long runs of tiny matmuls).
def _mm_ns(w):
    return w / 2.4 + 2.5


def _job_ns(cols):
    """PE-time of one 528-pass job with the given column count."""
    return 528.0 * sum(_mm_ns(w) for _, w in _c_blocks(cols))


def _build(A, B, TS):
    """Per-core Bass kernel: one merged job over the O (B cols,
    scaled), S (TS cols, unscaled shared) and R (A cols, scaled)
    column groups. Same NEFF on all cores."""
    nc = bacc.Bacc("TRN2", debug=False, enable_asserts=False,
                   num_devices=N_CORES, enable_partition_id=False)

    def din(name, shape, dt=BF16):
        return nc.dram_tensor(name, shape, dt, kind="ExternalInput").ap()

    def dout(name, shape, dt=BF16):
        return nc.dram_tensor(name, shape, dt, kind="ExternalOutput").ap()

    xr = din("xr", [128, DPO, A])            # routed tokens, [d_pi, d_po, c]
    xo = din("xo", [128, DPO, B])            # overflow tokens
    # no xs input: the shared-expert tokens are the first TS columns of
    # xr (the host assigns each token's shared computation to a core
    # whose routed slot already holds it, and orders xr accordingly)
    cwr = din("cwr", [128, A], F32)          # combine weights, replicated
    cwo = din("cwo", [128, B], F32)
    w1r = din("w1r", [IPO, 128, D])          # [i_blk][d_pi][d_po*128+i_c]
    w3r = din("w3r", [IPO, 128, D])
    w2r = din("w2r", [DPO, 128, I])          # [d_blk][i_pi][i_po*128+d_c]
    sw1t = din("sw1t", [IPO, 128, D])
    sw3t = din("sw3t", [IPO, 128, D])
    sw2t = din("sw2t", [DPO, 128, I])
    w1o = din("w1o", [IPO, 128, D])
    w3o = din("w3o", [IPO, 128, D])
    w2o = din("w2o", [DPO, 128, I])
    yr = dout("yr", [128, DPO, A])           # [d_pi, d_po, c]
    zs = dout("zs", [128, DPO, TS])
    yo = dout("yo", [128, DPO, B])

    Silu = mybir.ActivationFunctionType.Silu

    with TileContext(nc) as tc:
        with tc.tile_pool(name="main", bufs=1) as pool, \
             tc.tile_pool(name="psum", bufs=1, space="PSUM") as pp:
            cwr_sb = pool.tile([128, A], F32, tag="cwr", bufs=1, name="cwr_sb")
            cwo_sb = pool.tile([128, B], F32, tag="cwo", bufs=1, name="cwo_sb")

            # column groups, in phase-A execution order (small first so
            # the PE start is gated by the least DMA)
            xo_sb = pool.tile([128, DPO, B], BF16, tag="x_o", bufs=1,
                              name="x_o")
            xr_sb = pool.tile([128, DPO, A], BF16, tag="x_r", bufs=1,
                              name="x_r")
            Ho = pool.tile([128, IPO, B], BF16, tag="H_o", bufs=1, name="H_o")
            Hs = pool.tile([128, IPO, TS], BF16, tag="H_s", bufs=1,
                           name="H_s")
            Hr = pool.tile([128, IPO, A], BF16, tag="H_r", bufs=1, name="H_r")

            # groups: (name, w1_dram, w3_dram, w2_dram, x_sb tile,
            #          col blocks, H tile, cw tile, out dram)
            groups = [
                ("o", w1o, w3o, w2o, xo_sb, _c_blocks(B), Ho, cwo_sb, yo),
                ("s", sw1t, sw3t, sw2t, xr_sb, _c_blocks(TS), Hs, None, zs),
                ("r", w1r, w3r, w2r, xr_sb, _c_blocks(A), Hr, cwr_sb, yr),
            ]

            # ---- input DMAs, gating-first order. R's phase-A
            # sub-passes are shifted 2 iterations later than O/S so the
            # 4MB xr bulk and R weights get ~15us of extra deadline and
            # the startup wire isn't oversubscribed.
            RSHIFT = 2
            w13 = {}

            def w13_dma(gname, wd, wn, i, split=True):
                # always piecewise: the d=0 matmul then gates on the
                # first 131KB instead of the whole 512KB chunk
                w_sb = pool.tile([128, DPO, 128], BF16, tag="w13",
                                 bufs=14, name=f"{wn}_{gname}_{i}")
                src = wd[i].rearrange("p (a b) -> p a b", a=DPO)
                nc.sync.dma_start(w_sb[:, 0:4, :], src[:, 0:4, :])
                nc.sync.dma_start(w_sb[:, 4:10, :], src[:, 4:10, :])
                nc.sync.dma_start(w_sb[:, 10:, :], src[:, 10:, :])
                w13[(gname, wn, i)] = w_sb

            # S[0] runs first and gates only on its first d-slices +
            # the leading pieces of its weight chunks; everything else
            # streams under its ~7us of compute
            for dsl in range(0, 2):
                nc.sync.dma_start(xr_sb[:, dsl, 0:TS], xr[:, dsl, 0:TS])
            w13_dma("s", sw1t, "w1", 0)
            w13_dma("s", sw3t, "w3", 0)
            for dsl in range(2, 8):
                nc.sync.dma_start(xr_sb[:, dsl, 0:TS], xr[:, dsl, 0:TS])
            nc.sync.dma_start(xo_sb[:], xo[:])
            w13_dma("o", w1o, "w1", 0)
            w13_dma("o", w3o, "w3", 0)
            for dsl in range(8, DPO):
                nc.sync.dma_start(xr_sb[:, dsl, 0:TS], xr[:, dsl, 0:TS])
            w13_dma("s", sw1t, "w1", 1)
            w13_dma("s", sw3t, "w3", 1)
            w13_dma("r", w1r, "w1", 0)
            w13_dma("r", w3r, "w3", 0)

            # ---- phase A: H = silu(x@w1T) * (x@w3T) [* cw] ----
            def a_subpass(g, i):
                gname, w1_d, w3_d, _, x_sb, cbs, H, cw_sb, _ = g
                w1_sb = w13.pop((gname, "w1", i))
                w3_sb = w13.pop((gname, "w3", i))
                p1s = []
                p3s = []
                for bi, (off, w) in enumerate(cbs):
                    p1s.append(pp.tile([128, w], F32, tag="ps", bufs=8,
                                       name=f"p1_{gname}_{i}_{bi}"))
                    p3s.append(pp.tile([128, w], F32, tag="ps", bufs=8,
                                       name=f"p3_{gname}_{i}_{bi}"))
                for d in range(DPO):
                    for bi, (off, w) in enumerate(cbs):
                        nc.tensor.matmul(
                            p1s[bi][:], w1_sb[:, d, :],
                            x_sb[:, d, off:off + w],
                            start=(d == 0), stop=(d == DPO - 1))
                    for bi, (off, w) in enumerate(cbs):
                        nc.tensor.matmul(
                            p3s[bi][:], w3_sb[:, d, :],
                            x_sb[:, d, off:off + w],
                            start=(d == 0), stop=(d == DPO - 1))
                for bi, (off, w) in enumerate(cbs):
                    s_t = pool.tile([128, w], F32, tag="act1", bufs=6,
                                    name=f"s_{gname}_{i}_{bi}")
                    nc.scalar.activation(s_t[:], p1s[bi][:], Silu)
                    # cw is per-token (per-column), so it commutes with
                    # the phase-B contraction: applied there instead
                    nc.vector.tensor_mul(H[:, i, off:off + w],
                                         s_t[:], p3s[bi][:])

            # per-iteration weight-issue schedule: each chunk enters
            # the DMA queue just-in-time (~2 iterations of lead), so the
            # startup transient isn't oversubscribed
            from collections import defaultdict
            iss = defaultdict(list)
            iss[1].append(("o", 1))
            for i in range(2, IPO):
                iss[i - 2].append(("s", i))
            for i in range(2, IPO):
                iss[i + 1].append(("o", i))
            for i in range(1, IPO):
                iss[i + 1].append(("r", i))
            wsrc = {"o": (w1o, w3o), "s": (sw1t, sw3t), "r": (w1r, w3r)}

            # schedule: O[0] first (tiny PE warm-up while x streams),
            # then O shifted OSHIFT late (its weight stream is 1MB per
            # 0.8us of PE work - keep it out of the startup transient),
            # S unshifted, R shifted RSHIFT (xr bulk arrives JIT)
            go, gs, gr = groups
            OSHIFT, RSHIFT = 3, 2
            for j in range(IPO + OSHIFT):
                for gname, i in iss.get(j, ()):
                    w1_d, w3_d = wsrc[gname]
                    w13_dma(gname, w1_d, "w1", i)
                    w13_dma(gname, w3_d, "w3", i)
                if j < IPO:
                    a_subpass(gs, j)
                if j == 0:
                    a_subpass(go, 0)
                oi = j - OSHIFT
                if 1 <= oi < IPO:
                    a_subpass(go, oi)
                if j < 3:
                    span = A - TS
                    m = TS + span // 2
                    lo, hi = ((0, 6), (6, 11), (11, DPO))[j]
                    for dsl in range(lo, hi):
                        nc.sync.dma_start(xr_sb[:, dsl, TS:m],
                                          xr[:, dsl, TS:m])
                        nc.sync.dma_start(xr_sb[:, dsl, m:],
                                          xr[:, dsl, m:])
                ri = j - RSHIFT
                if 0 <= ri < IPO:
                    a_subpass(gr, ri)

            # cw tensors are only read by phase-B output multiplies
            nc.sync.dma_start(cwr_sb[:], cwr[:])
            nc.sync.dma_start(cwo_sb[:], cwo[:])

            # ---- phase B: out = H @ w2T  (R first, O last: tiny tail) ----
            groups_b = [groups[2], groups[1], groups[0]]
            for do in range(DPO):
                w2s = {}
                for gname, _, _, w2_d, _, _, _, _, _ in groups_b:
                    w2_sb = pool.tile([128, IPO, 128], BF16, tag="w2",
                                      bufs=6, name=f"w2_{gname}_{do}")
                    nc.sync.dma_start(
                        w2_sb[:], w2_d[do].rearrange("p (a b) -> p a b",
                                                     a=IPO))
                    w2s[gname] = w2_sb
                for gname, _, _, _, _, cbs, H, cw_sb, out_d in groups_b:
                    pys = []
                    for bi, (off, w) in enumerate(cbs):
                        pys.append(pp.tile([128, w], F32, tag="ps", bufs=8,
                                           name=f"py_{gname}_{do}_{bi}"))
                    for i in range(IPO):
                        for bi, (off, w) in enumerate(cbs):
                            nc.tensor.matmul(
                                pys[bi][:], w2s[gname][:, i, :],
                                H[:, i, off:off + w],
                                start=(i == 0), stop=(i == IPO - 1))
                    for bi, (off, w) in enumerate(cbs):
                        y_t = pool.tile([128, w], BF16, tag="yo", bufs=8,
                                        name=f"y_{gname}_{do}_{bi}")
                        if cw_sb is not None:
                            nc.vector.tensor_mul(y_t[:], pys[bi][:],
                                                 cw_sb[:, off:off + w])
                        else:
                            nc.vector.tensor_copy(y_t[:], pys[bi][:])
                        nc.sync.dma_start(out_d[:, do, off:off + w], y_t[:])

    nc.finalize()
    return nc


def _get_kernel(A, B, TS):
    key = (A, B, TS)
    if key not in _BUILD_CACHE:
        _BUILD_CACHE[key] = _build(A, B, TS)
    return _BUILD_CACHE[key]


def _solve_sizes(counts):
    """Choose (a, b) minimizing modeled per-core PE time of the R and O
    work, subject to the overflow pieces fitting in 8 one-expert slots
    of b columns each."""
    cmin, cmax = int(counts.min()), int(counts.max())
    best = None
    for a in range(max(cmin - 192, 1), cmax + 1):
        o = [int(c) - a for c in counts if c > a]
        if not o:
            b = 16  # degenerate: no overflow at all
        else:
            # minimal b with sum(ceil(o/b)) <= 8
            b = max(16, -(-sum(o) // 8))
            while sum(-(-v // b) for v in o) > 8:
                b += 1
        cost = _job_ns(a) + 528.0 * _mm_ns(b)
        if best is None or cost < best[0]:
            best = (cost, a, b)
    _, a, b = best
    return a, min(max(b, 16), 512)


def _shared_assign(tok_lists, counts, A, T, TS):
    """Assign each token's shared-expert compute to a core whose R slot
    holds it, with exactly TS tokens per core (BFS-chain balancing of a
    2-candidate orientation). Returns None if infeasible."""
    cand = [[] for _ in range(T)]
    for e in range(E):
        for t in tok_lists[e]:
            cand[int(t)].append(e)
    if any(not c for c in cand):
        return None
    load = [0] * E
    assign = np.full(T, -1, np.int32)
    for t in sorted(range(T), key=lambda t: len(cand[t])):
        k = min(cand[t], key=lambda c: load[c])
        assign[t] = k
        load[k] += 1
    # tokens movable from core k to the other candidate
    from collections import defaultdict, deque
    out_edges = defaultdict(list)  # core -> [(token, other_core)]
    for t in range(T):
        for c in cand[t]:
            if c != assign[t]:
                out_edges[int(assign[t])].append((t, c))
    # rebuild adjacency lazily inside BFS instead (loads are small)
    def bfs_fix(src_core):
        # find chain src_core -> ... -> underfull core, moving one token
        # along each hop
        prev = {src_core: None}
        qd = deque([src_core])
        while qd:
            k = qd.popleft()
            if load[k] < TS and k != src_core:
                # unwind: move tokens along the chain
                cur = k
                path = []
                while prev[cur] is not None:
                    pk, tok = prev[cur]
                    path.append((tok, pk, cur))
                    cur = pk
                for tok, a_, b_ in reversed(path):
                    assign[tok] = b_
                    load[a_] -= 1
                    load[b_] += 1
                return True
            for t in range(T):
                if int(assign[t]) != k:
                    continue
                for c in cand[t]:
                    if c != k and c not in prev:
                        prev[c] = (k, t)
                        qd.append(c)
        return False
    guard = 0
    while any(l > TS for l in load):
        src_core = max(range(E), key=lambda k: load[k])
        if not bfs_fix(src_core):
            return None
        guard += 1
        if guard > 4 * T:
            return None
    if any(l != TS for l in load):
        return None
    return assign


def _pm(a, po):
    """[N, po*128] -> partition-major [128, po, N] contiguous."""
    n = a.shape[0]
    return np.ascontiguousarray(
        a.T.reshape(po, 128, n).transpose(1, 0, 2))


def kernel(x, gate_w, gate_b, w1, w2, w3, sw1, sw2, sw3):
    bf16 = ml_dtypes.bfloat16
    x = np.asarray(x)
    gate_w = np.asarray(gate_w, dtype=np.float32)
    gate_b = np.asarray(gate_b, dtype=np.float32)
    w1 = np.asarray(w1)
    w2 = np.asarray(w2)
    w3 = np.asarray(w3)
    sw1 = np.asarray(sw1)
    sw2 = np.asarray(sw2)
    sw3 = np.asarray(sw3)

    B_, S_, Dx = x.shape
    assert Dx == D
    T = B_ * S_
    TS = T // N_CORES
    xt = x.reshape(T, D)

    # ---- gate (fp32, mirrors reference: sqrt(softplus), top-2 on biased) ----
    xf = xt.astype(np.float32)
    logits = xf @ gate_w.T
    scores = np.sqrt(np.log1p(np.exp(-np.abs(logits)))
                     + np.maximum(logits, 0.0))
    biased = scores + gate_b
    idx = np.argsort(-biased, axis=1, kind="stable")[:, :TOPK]
    cw = np.zeros((T, E), dtype=np.float32)
    np.put_along_axis(cw, idx, np.take_along_axis(scores, idx, axis=1), axis=1)

    sel = np.zeros((T, E), dtype=bool)
    np.put_along_axis(sel, idx, True, axis=1)
    tok_lists = [np.nonzero(sel[:, e])[0] for e in range(E)]
    counts = np.array([len(t) for t in tok_lists])

    A, Bb = _solve_sizes(counts)

    # ---- overflow selection: for each expert with more than A tokens,
    # pick the excess from tokens NOT overflowed in their other expert,
    # so every token keeps at least one xr-resident instance
    overflowed = set()
    ov_sets = {}
    for e in np.argsort(-counts):
        need = int(counts[e]) - A
        if need <= 0:
            continue
        pick = []
        for t in tok_lists[int(e)][::-1]:
            if need == 0:
                break
            if int(t) not in overflowed:
                pick.append(int(t))
                need -= 1
        if need > 0:
            raise RuntimeError("overflow selection infeasible")
        ov_sets[int(e)] = set(pick)
        overflowed.update(pick)
    resident = []
    for e in range(E):
        ov = ov_sets.get(e, ())
        resident.append(np.array([t for t in tok_lists[e]
                                  if int(t) not in ov], dtype=np.int64))

    # ---- shared-token assignment: each token's shared-expert compute
    # runs on a core whose R slot already holds it (top-2 routing gives
    # every token up to two candidate cores); loads must hit TS exactly
    assign = _shared_assign(resident, counts, A, T, TS)
    if assign is None:
        raise RuntimeError(
            "shared-token flow infeasible for this routing; "
            "unsupported input distribution")

    # ---- O-slot assignment: split overflows into <=Bb-col pieces ----
    slots = [None] * N_CORES  # (expert, tok_idx array)
    free = list(range(N_CORES))
    for e in np.argsort(-counts):
        ov = np.array(sorted(ov_sets.get(int(e), ())), dtype=np.int64)
        pos = 0
        while pos < len(ov):
            take = min(len(ov) - pos, Bb)
            k = free.pop(0)
            slots[k] = (int(e), ov[pos:pos + take])
            pos += take

    nc = _get_kernel(A, Bb, TS)

    # ---- per-core input prep ----
    def wA_layout(wm):  # [I, D] -> [IPO, 128, D]; [ib,pi,po*128+ic]
        return np.ascontiguousarray(
            wm.T.reshape(DPO, 128, IPO, 128).transpose(2, 1, 0, 3)
        ).reshape(IPO, 128, D)

    def wB_layout(wm):  # [D, I] -> [DPO, 128, I]; [db,pi,po*128+dc]
        return np.ascontiguousarray(
            wm.T.reshape(IPO, 128, DPO, 128).transpose(2, 1, 0, 3)
        ).reshape(DPO, 128, I)

    w1L = [wA_layout(w1[e]) for e in range(E)]
    w3L = [wA_layout(w3[e]) for e in range(E)]
    w2L = [wB_layout(w2[e]) for e in range(E)]
    sw1L = wA_layout(sw1)
    sw3L = wA_layout(sw3)
    sw2L = wB_layout(sw2)

    in_maps = []
    r_toks = []
    s_toks = []
    for k in range(N_CORES):
        toks = resident[k][:A]
        # shared-assigned tokens first, rest after (kernel computes the
        # shared expert over xr columns 0:TS)
        is_sh = assign[toks] == k
        toks = np.concatenate([toks[is_sh], toks[~is_sh]])
        assert is_sh.sum() == TS
        cnt = len(toks)
        r_toks.append(toks)
        s_toks.append(toks[:TS])
        xg = np.zeros((A, D), dtype=bf16)
        xg[:cnt] = xt[toks]
        cwe = np.zeros((A,), dtype=np.float32)
        cwe[:cnt] = cw[toks, k]

        if slots[k] is not None:
            oe, otoks = slots[k]
        else:
            oe, otoks = k, np.zeros((0,), dtype=np.int64)
        ocnt = len(otoks)
        xg_o = np.zeros((Bb, D), dtype=bf16)
        xg_o[:ocnt] = xt[otoks]
        cwe_o = np.zeros((Bb,), dtype=np.float32)
        cwe_o[:ocnt] = cw[otoks, oe]

        in_maps.append({
            "xr": _pm(xg, DPO),
            "xo": _pm(xg_o, DPO),
            "cwr": np.ascontiguousarray(
                np.broadcast_to(cwe[None, :], (128, A))),
            "cwo": np.ascontiguousarray(
                np.broadcast_to(cwe_o[None, :], (128, Bb))),
            "w1r": w1L[k], "w3r": w3L[k], "w2r": w2L[k],
            "w1o": w1L[oe], "w3o": w3L[oe], "w2o": w2L[oe],
            "sw1t": sw1L, "sw3t": sw3L, "sw2t": sw2L,
        })

    res = bass_utils.run_bass_kernel_spmd(
        nc, in_maps, core_ids=list(range(N_CORES)))
    global LAST_RESULT
    LAST_RESULT = res

    # ---- unshard + combine (bf16, reference expert order) ----
    y = np.zeros((T, D), dtype=bf16)
    for e in range(E):
        acc_toks = []
        acc_vals = []
        toks = r_toks[e]
        ye = res.results[e]["yr"]                       # [128, DPO, A]
        ye_tok = ye.transpose(2, 1, 0).reshape(A, D)    # [c, d]
        acc_toks.append(toks)
        acc_vals.append(ye_tok[:len(toks)])
        for k in range(N_CORES):
            if slots[k] is not None and slots[k][0] == e and len(slots[k][1]):
                yo = res.results[k]["yo"]
                yo_tok = yo.transpose(2, 1, 0).reshape(Bb, D)
                acc_toks.append(slots[k][1])
                acc_vals.append(yo_tok[:len(slots[k][1])])
        at = np.concatenate(acc_toks)
        av = np.concatenate(acc_vals, axis=0)
        y[at] = y[at] + av
    z = np.zeros((T, D), dtype=bf16)
    for k in range(N_CORES):
        zk = res.results[k]["zs"].transpose(2, 1, 0).reshape(TS, D)
        z[s_toks[k]] = zk
    out = (y + z).reshape(B_, S_, D)
    return out.astype(x.dtype)



# revision 5
# speedup vs baseline: 1.0442x; 1.0442x over previous
"""MoE (8 routed experts, top-2, + shared expert) on 8 TRN2 NeuronCores.

Strategy: expert-parallel, no weight duplication. Host computes the
gate (fp32 numpy, mirroring the reference), then each core k runs ONE
static SPMD kernel over two column groups:

  R (A cols):  ALL tokens routed to expert k (A = max expert count;
               cores with fewer tokens run zero-padded columns)
  S (TS cols): a 1/8 token-slice of the shared expert, chosen to be a
               subset of the core's R tokens so xr is loaded once

Earlier revisions load-balanced the routed experts with an "overflow"
column group bound to a second expert per core.  That cut PE columns
~5% but forced every core to stream a second expert's full weight set
(+17.3MB, +33% HBM traffic).  Profiling showed the kernel's real
limiter is per-core DVFS throttling driven by total activity: cores
drop to ~81% PE clock under the combined matmul+DMA load, and the
slowest (most-throttled) core sets the SPMD exec time.  Trading the
overflow group's HBM traffic for padded columns keeps the wire at
~110GB/s (vs ~150) and lets the PE run unthrottled more of the time.

All device tensors are pre-arranged on host into partition-major
layouts so every DMA is contiguous per partition: matmul lhsT/rhs
always have the contraction dim chunked as [pi=128, po, free].
"""

import numpy as np
import ml_dtypes

import concourse.mybir as mybir
from concourse import bacc
from concourse.tile import TileContext
from concourse import bass_utils

BF16 = mybir.dt.bfloat16
F32 = mybir.dt.float32

D = 2048          # model dim
I = 1408          # expert inter dim
E = 8             # routed experts
TOPK = 2
N_CORES = 8
DPO = D // 128    # 16 chunks of the model dim
IPO = I // 128    # 11 chunks of the inter dim

_BUILD_CACHE = {}


def _c_blocks(C):
    """Split C columns into equal-ish blocks <= 512."""
    nb = -(-C // 512)
    per = -(-C // (nb * 128)) * 128
    blocks = []
    off = 0
    while off < C:
        w = min(per, C - off)
        blocks.append((off, w))
        off += w
    return blocks


def _build(A, TS):
    """Per-core Bass kernel: S (TS cols, unscaled shared) and R (A
    cols, scaled by cw) column groups. Same NEFF on all cores."""
    nc = bacc.Bacc("TRN2", debug=False, enable_asserts=False,
                   num_devices=N_CORES, enable_partition_id=False)

    def din(name, shape, dt=BF16):
        return nc.dram_tensor(name, shape, dt, kind="ExternalInput").ap()

    def dout(name, shape, dt=BF16):
        return nc.dram_tensor(name, shape, dt, kind="ExternalOutput").ap()

    xr = din("xr", [128, DPO, A])            # routed tokens, [d_pi, d_po, c]
    # no xs input: the shared-expert tokens are the first TS columns of
    # xr (the host assigns each token's shared computation to a core
    # whose routed slot already holds it, and orders xr accordingly)
    cwr = din("cwr", [128, A], F32)          # combine weights, replicated
    w1r = din("w1r", [IPO, 128, D])          # [i_blk][d_pi][d_po*128+i_c]
    w3r = din("w3r", [IPO, 128, D])
    w2r = din("w2r", [DPO, 128, I])          # [d_blk][i_pi][i_po*128+d_c]
    sw1t = din("sw1t", [IPO, 128, D])
    sw3t = din("sw3t", [IPO, 128, D])
    sw2t = din("sw2t", [DPO, 128, I])
    yr = dout("yr", [128, DPO, A])           # [d_pi, d_po, c]
    zs = dout("zs", [128, DPO, TS])

    Silu = mybir.ActivationFunctionType.Silu

    with TileContext(nc) as tc:
        with tc.tile_pool(name="main", bufs=1) as pool, \
             tc.tile_pool(name="psum", bufs=1, space="PSUM") as pp:
            cwr_sb = pool.tile([128, A], F32, tag="cwr", bufs=1, name="cwr_sb")

            xr_sb = pool.tile([128, DPO, A], BF16, tag="x_r", bufs=1,
                              name="x_r")
            Hs = pool.tile([128, IPO, TS], BF16, tag="H_s", bufs=1,
                           name="H_s")
            Hr = pool.tile([128, IPO, A], BF16, tag="H_r", bufs=1, name="H_r")

            # groups: (name, w1_dram, w3_dram, w2_dram, x_sb tile,
            #          col blocks, H tile, cw tile, out dram)
            groups = [
                ("s", sw1t, sw3t, sw2t, xr_sb, _c_blocks(TS), Hs, None, zs),
                ("r", w1r, w3r, w2r, xr_sb, _c_blocks(A), Hr, cwr_sb, yr),
            ]

            w13 = {}

            def w13_dma(gname, wd, wn, i):
                # piecewise: the d=0 matmul then gates on the first
                # 131KB instead of the whole 512KB chunk
                w_sb = pool.tile([128, DPO, 128], BF16, tag="w13",
                                 bufs=14, name=f"{wn}_{gname}_{i}")
                src = wd[i].rearrange("p (a b) -> p a b", a=DPO)
                nc.sync.dma_start(w_sb[:, 0:4, :], src[:, 0:4, :])
                nc.sync.dma_start(w_sb[:, 4:10, :], src[:, 4:10, :])
                nc.sync.dma_start(w_sb[:, 10:, :], src[:, 10:, :])
                w13[(gname, wn, i)] = w_sb

            # ---- input DMAs, gating-first order. S[0] runs first and
            # gates only on its first d-slices + the leading pieces of
            # its weight chunks; everything else streams under its
            # ~7us of compute.
            for dsl in range(0, 2):
                nc.sync.dma_start(xr_sb[:, dsl, 0:TS], xr[:, dsl, 0:TS])
            w13_dma("s", sw1t, "w1", 0)
            w13_dma("s", sw3t, "w3", 0)
            for dsl in range(2, 8):
                nc.sync.dma_start(xr_sb[:, dsl, 0:TS], xr[:, dsl, 0:TS])
            for dsl in range(8, DPO):
                nc.sync.dma_start(xr_sb[:, dsl, 0:TS], xr[:, dsl, 0:TS])
            w13_dma("s", sw1t, "w1", 1)
            w13_dma("s", sw3t, "w3", 1)
            w13_dma("r", w1r, "w1", 0)
            w13_dma("r", w3r, "w3", 0)

            # ---- phase A: H = silu(x@w1T) * (x@w3T) ----
            def a_subpass(g, i):
                gname, w1_d, w3_d, _, x_sb, cbs, H, cw_sb, _ = g
                w1_sb = w13.pop((gname, "w1", i))
                w3_sb = w13.pop((gname, "w3", i))
                p1s = []
                p3s = []
                for bi, (off, w) in enumerate(cbs):
                    p1s.append(pp.tile([128, w], F32, tag="ps", bufs=8,
                                       name=f"p1_{gname}_{i}_{bi}"))
                    p3s.append(pp.tile([128, w], F32, tag="ps", bufs=8,
                                       name=f"p3_{gname}_{i}_{bi}"))
                for d in range(DPO):
                    for bi, (off, w) in enumerate(cbs):
                        nc.tensor.matmul(
                            p1s[bi][:], w1_sb[:, d, :],
                            x_sb[:, d, off:off + w],
                            start=(d == 0), stop=(d == DPO - 1))
                    for bi, (off, w) in enumerate(cbs):
                        nc.tensor.matmul(
                            p3s[bi][:], w3_sb[:, d, :],
                            x_sb[:, d, off:off + w],
                            start=(d == 0), stop=(d == DPO - 1))
                for bi, (off, w) in enumerate(cbs):
                    s_t = pool.tile([128, w], F32, tag="act1", bufs=6,
                                    name=f"s_{gname}_{i}_{bi}")
                    nc.scalar.activation(s_t[:], p1s[bi][:], Silu)
                    # cw is per-token (per-column), so it commutes with
                    # the phase-B contraction: applied there instead
                    nc.vector.tensor_mul(H[:, i, off:off + w],
                                         s_t[:], p3s[bi][:])

            # per-iteration weight-issue schedule: each chunk enters
            # the DMA queue just-in-time (~2 iterations of lead), so
            # the startup transient isn't oversubscribed
            from collections import defaultdict
            iss = defaultdict(list)
            for i in range(2, IPO):
                iss[i - 2].append(("s", i))
            for i in range(1, IPO):
                iss[i].append(("r", i))
            wsrc = {"s": (sw1t, sw3t), "r": (w1r, w3r)}

            # schedule: S unshifted, R shifted RSHIFT (xr bulk arrives
            # just-in-time under S's leading compute)
            gs, gr = groups
            RSHIFT = 2
            for j in range(IPO + RSHIFT):
                for gname, i in iss.get(j, ()):
                    w1_d, w3_d = wsrc[gname]
                    w13_dma(gname, w1_d, "w1", i)
                    w13_dma(gname, w3_d, "w3", i)
                if j < IPO:
                    a_subpass(gs, j)
                if j < 3:
                    span = A - TS
                    m = TS + span // 2
                    lo, hi = ((0, 6), (6, 11), (11, DPO))[j]
                    for dsl in range(lo, hi):
                        nc.sync.dma_start(xr_sb[:, dsl, TS:m],
                                          xr[:, dsl, TS:m])
                        nc.sync.dma_start(xr_sb[:, dsl, m:],
                                          xr[:, dsl, m:])
                ri = j - RSHIFT
                if 0 <= ri < IPO:
                    a_subpass(gr, ri)

            # cw tensor is only read by phase-B output multiplies
            nc.sync.dma_start(cwr_sb[:], cwr[:])

            # ---- phase B: out = H @ w2T  (R first, S last: small tail) ----
            groups_b = [groups[1], groups[0]]
            for do in range(DPO):
                w2s = {}
                for gname, _, _, w2_d, _, _, _, _, _ in groups_b:
                    w2_sb = pool.tile([128, IPO, 128], BF16, tag="w2",
                                      bufs=6, name=f"w2_{gname}_{do}")
                    nc.sync.dma_start(
                        w2_sb[:], w2_d[do].rearrange("p (a b) -> p a b",
                                                     a=IPO))
                    w2s[gname] = w2_sb
                for gname, _, _, _, _, cbs, H, cw_sb, out_d in groups_b:
                    pys = []
                    for bi, (off, w) in enumerate(cbs):
                        pys.append(pp.tile([128, w], F32, tag="ps", bufs=8,
                                           name=f"py_{gname}_{do}_{bi}"))
                    for i in range(IPO):
                        for bi, (off, w) in enumerate(cbs):
                            nc.tensor.matmul(
                                pys[bi][:], w2s[gname][:, i, :],
                                H[:, i, off:off + w],
                                start=(i == 0), stop=(i == IPO - 1))
                    for bi, (off, w) in enumerate(cbs):
                        y_t = pool.tile([128, w], BF16, tag="yo", bufs=8,
                                        name=f"y_{gname}_{do}_{bi}")
                        if cw_sb is not None:
                            nc.vector.tensor_mul(y_t[:], pys[bi][:],
                                                 cw_sb[:, off:off + w])
                        else:
                            nc.vector.tensor_copy(y_t[:], pys[bi][:])
                        nc.sync.dma_start(out_d[:, do, off:off + w], y_t[:])

    nc.finalize()
    return nc


def _get_kernel(A, TS):
    key = (A, TS)
    if key not in _BUILD_CACHE:
        _BUILD_CACHE[key] = _build(A, TS)
    return _BUILD_CACHE[key]


def _shared_assign(tok_lists, T, TS):
    """Assign each token's shared-expert compute to a core whose R slot
    holds it, with exactly TS tokens per core (BFS-chain balancing of a
    2-candidate orientation). Returns None if infeasible."""
    cand = [[] for _ in range(T)]
    for e in range(E):
        for t in tok_lists[e]:
            cand[int(t)].append(e)
    if any(not c for c in cand):
        return None
    load = [0] * E
    assign = np.full(T, -1, np.int32)
    for t in sorted(range(T), key=lambda t: len(cand[t])):
        k = min(cand[t], key=lambda c: load[c])
        assign[t] = k
        load[k] += 1
    from collections import deque

    def bfs_fix(src_core):
        # find chain src_core -> ... -> underfull core, moving one
        # token along each hop
        prev = {src_core: None}
        qd = deque([src_core])
        while qd:
            k = qd.popleft()
            if load[k] < TS and k != src_core:
                cur = k
                path = []
                while prev[cur] is not None:
                    pk, tok = prev[cur]
                    path.append((tok, pk, cur))
                    cur = pk
                for tok, a_, b_ in reversed(path):
                    assign[tok] = b_
                    load[a_] -= 1
                    load[b_] += 1
                return True
            for t in range(T):
                if int(assign[t]) != k:
                    continue
                for c in cand[t]:
                    if c != k and c not in prev:
                        prev[c] = (k, t)
                        qd.append(c)
        return False

    guard = 0
    while any(l > TS for l in load):
        src_core = max(range(E), key=lambda k: load[k])
        if not bfs_fix(src_core):
            return None
        guard += 1
        if guard > 4 * T:
            return None
    if any(l != TS for l in load):
        return None
    return assign


def _pm(a, po):
    """[N, po*128] -> partition-major [128, po, N] contiguous."""
    n = a.shape[0]
    return np.ascontiguousarray(
        a.T.reshape(po, 128, n).transpose(1, 0, 2))


def kernel(x, gate_w, gate_b, w1, w2, w3, sw1, sw2, sw3):
    bf16 = ml_dtypes.bfloat16
    x = np.asarray(x)
    gate_w = np.asarray(gate_w, dtype=np.float32)
    gate_b = np.asarray(gate_b, dtype=np.float32)
    w1 = np.asarray(w1)
    w2 = np.asarray(w2)
    w3 = np.asarray(w3)
    sw1 = np.asarray(sw1)
    sw2 = np.asarray(sw2)
    sw3 = np.asarray(sw3)

    B_, S_, Dx = x.shape
    assert Dx == D
    T = B_ * S_
    TS = T // N_CORES
    xt = x.reshape(T, D)

    # ---- gate (fp32, mirrors reference: sqrt(softplus), top-2 on biased) ----
    xf = xt.astype(np.float32)
    logits = xf @ gate_w.T
    scores = np.sqrt(np.log1p(np.exp(-np.abs(logits)))
                     + np.maximum(logits, 0.0))
    biased = scores + gate_b
    idx = np.argsort(-biased, axis=1, kind="stable")[:, :TOPK]
    cw = np.zeros((T, E), dtype=np.float32)
    np.put_along_axis(cw, idx, np.take_along_axis(scores, idx, axis=1), axis=1)

    sel = np.zeros((T, E), dtype=bool)
    np.put_along_axis(sel, idx, True, axis=1)
    tok_lists = [np.nonzero(sel[:, e])[0] for e in range(E)]
    counts = np.array([len(t) for t in tok_lists])

    A = int(counts.max())

    # ---- shared-token assignment: each token's shared-expert compute
    # runs on a core whose R slot already holds it (top-2 routing gives
    # every token two candidate cores); loads must hit TS exactly
    assign = _shared_assign(tok_lists, T, TS)
    if assign is None:
        raise RuntimeError(
            "shared-token flow infeasible for this routing; "
            "unsupported input distribution")

    nc = _get_kernel(A, TS)

    # ---- per-core input prep ----
    def wA_layout(wm):  # [I, D] -> [IPO, 128, D]; [ib,pi,po*128+ic]
        return np.ascontiguousarray(
            wm.T.reshape(DPO, 128, IPO, 128).transpose(2, 1, 0, 3)
        ).reshape(IPO, 128, D)

    def wB_layout(wm):  # [D, I] -> [DPO, 128, I]; [db,pi,po*128+dc]
        return np.ascontiguousarray(
            wm.T.reshape(IPO, 128, DPO, 128).transpose(2, 1, 0, 3)
        ).reshape(DPO, 128, I)

    sw1L = wA_layout(sw1)
    sw3L = wA_layout(sw3)
    sw2L = wB_layout(sw2)

    in_maps = []
    r_toks = []
    s_toks = []
    for k in range(N_CORES):
        toks = tok_lists[k]
        # shared-assigned tokens first, rest after (kernel computes the
        # shared expert over xr columns 0:TS)
        is_sh = assign[toks] == k
        toks = np.concatenate([toks[is_sh], toks[~is_sh]])
        assert is_sh.sum() == TS
        cnt = len(toks)
        r_toks.append(toks)
        s_toks.append(toks[:TS])
        xg = np.zeros((A, D), dtype=bf16)
        xg[:cnt] = xt[toks]
        cwe = np.zeros((A,), dtype=np.float32)
        cwe[:cnt] = cw[toks, k]

        in_maps.append({
            "xr": _pm(xg, DPO),
            "cwr": np.ascontiguousarray(
                np.broadcast_to(cwe[None, :], (128, A))),
            "w1r": wA_layout(w1[k]), "w3r": wA_layout(w3[k]),
            "w2r": wB_layout(w2[k]),
            "sw1t": sw1L, "sw3t": sw3L, "sw2t": sw2L,
        })

    res = bass_utils.run_bass_kernel_spmd(
        nc, in_maps, core_ids=list(range(N_CORES)))
    global LAST_RESULT
    LAST_RESULT = res

    # ---- unshard + combine (bf16, reference expert order) ----
    y = np.zeros((T, D), dtype=bf16)
    for e in range(E):
        toks = r_toks[e]
        ye = res.results[e]["yr"]                       # [128, DPO, A]
        ye_tok = ye.transpose(2, 1, 0).reshape(A, D)    # [c, d]
        y[toks] = y[toks] + ye_tok[:len(toks)]
    z = np.zeros((T, D), dtype=bf16)
    for k in range(N_CORES):
        zk = res.results[k]["zs"].transpose(2, 1, 0).reshape(TS, D)
        z[s_toks[k]] = zk
    out = (y + z).reshape(B_, S_, D)
    return out.astype(x.dtype)
